# revision 11
# baseline (speedup 1.0000x reference)
"""Trainium2 Bass kernel for PopulationAttention (attention without softmax).

Math:  out = (Q @ K^T / sqrt(64)) @ (V * x)
Since there is no softmax, matmul associativity applies:
       out = Q @ (K^T @ (V * x)) / 8
which collapses the O(N^2 D) computation into two O(N D^2) matmuls and
makes the problem DMA-bound (~16 MB of HBM traffic per core).

Sharding: B*H = 64 (batch, head) pairs -> 8 pairs per core.
Core i handles batch b = i//2, heads h = (i%2)*8 .. (i%2)*8+8.

Per-core SBUF layout trick: an HBM [2048, 64] tensor is loaded as one
fully-contiguous 512KB DMA into a [128, 1024] SBUF tile, which implicitly
maps row n = p*16 + c to partition p, columns [64c, 64c+64).  The n-order
interleaving is harmless: stage 1 sums over all n, and stage 2 / the output
DMA use the same mapping consistently.
"""

import sys
import numpy as np

for _p in ("/opt/trn_rl_repo", "/root/.axon_site/_ro/trn_rl_repo"):
    if _p not in sys.path:
        sys.path.insert(0, _p)

B, H, N, D = 4, 16, 2048, 64
PAIRS = 8          # (b,h) pairs per core
NCORES = 8
C = N // 128       # 16 column-chunks of 64 in the [128, 1024] layout
SCALE = 1.0 / (D ** 0.5)  # 1/8, folded into x

_NC = None  # compiled Bass module cache


def _build(reps=1):
    import concourse.bass as bass
    import concourse.mybir as mybir
    import concourse.tile as tile
    from concourse import bacc
    from concourse.masks import make_identity

    f32 = mybir.dt.float32
    nc = bacc.Bacc("TRN2")

    Qd = nc.declare_dram_parameter("q", [PAIRS, N, D], f32, isOutput=False)
    Kd = nc.declare_dram_parameter("k", [PAIRS, N, D], f32, isOutput=False)
    Vd = nc.declare_dram_parameter("v", [PAIRS, N, D], f32, isOutput=False)
    Xd = nc.declare_dram_parameter("x", [N], f32, isOutput=False)
    Od = nc.declare_dram_parameter("out", [PAIRS, N, D], f32, isOutput=True)

    with tile.TileContext(nc) as tc:
        with (
            tc.tile_pool(name="const", bufs=1) as const,
            tc.tile_pool(name="kvq", bufs=3) as kvq,
            tc.tile_pool(name="vx", bufs=2) as vxp,
            tc.tile_pool(name="qt", bufs=3) as qtp,
            tc.tile_pool(name="kvs", bufs=2) as kvsp,
            tc.tile_pool(name="outs", bufs=2) as outp,
            tc.tile_pool(name="ps_kv", bufs=2, space="PSUM") as ps_kv,
            tc.tile_pool(name="ps_qt", bufs=2, space="PSUM") as ps_qt,
            tc.tile_pool(name="ps_o", bufs=2, space="PSUM") as ps_o,
        ):
            ident = const.tile([128, 128], f32)
            make_identity(nc, ident)

            xs = const.tile([128, C], f32)
            nc.sync.dma_start(out=xs, in_=Xd.rearrange("(p c) -> p c", p=128))
            nc.scalar.mul(xs, xs, SCALE)
            # expand to [128, C*D] so the per-pair multiply is a plain 2D
            # TensorTensor (3D broadcast APs overflow the TT sync-wait encoding)
            xs_e = const.tile([128, C * D], f32)
            for c in range(C):
                nc.scalar.copy(xs_e[:, c * D:(c + 1) * D],
                               xs[:, c:c + 1].broadcast_to([128, D]))

            for j in [jj for _ in range(reps) for jj in range(PAIRS)]:
                k_s = kvq.tile([128, C * D], f32, tag="k")
                nc.sync.dma_start(out=k_s, in_=Kd[j].rearrange("(p c) d -> p (c d)", p=128))
                v_s = kvq.tile([128, C * D], f32, tag="v")
                nc.sync.dma_start(out=v_s, in_=Vd[j].rearrange("(p c) d -> p (c d)", p=128))
                q_s = kvq.tile([128, C * D], f32, tag="q")
                nc.sync.dma_start(out=q_s, in_=Qd[j].rearrange("(p c) d -> p (c d)", p=128))

                # Vx[p, c*64+d] = V[p, c*64+d] * x[p, c] * 0.125
                vx = vxp.tile([128, C * D], f32)
                nc.vector.tensor_mul(vx, v_s, xs_e)

                # stage 1: KV[e, d] = sum_n K[n, e] * Vx[n, d]   (PSUM accum)
                kv_ps = ps_kv.tile([64, 64], f32)
                for c in range(C):
                    nc.tensor.matmul(
                        kv_ps,
                        k_s[:, c * D:(c + 1) * D],
                        vx[:, c * D:(c + 1) * D],
                        start=(c == 0),
                        stop=(c == C - 1),
                    )
                # replicate KV into both partition halves so stage-2 matmuls
                # can use lhsT slices based at partition 0 or 64
                kv_s = kvsp.tile([128, 64], f32)
                nc.vector.tensor_copy(kv_s[0:64, :], kv_ps)
                nc.vector.tensor_copy(kv_s[64:128, :], kv_ps)

                out_s = outp.tile([128, C * D], f32)
                # stage 2, per 128-column group t: transpose Q tile, then
                # out[n, d] = sum_e Q[n, e] KV[e, d] for the two chunks in it
                for t in range(C // 2):
                    qt_ps = ps_qt.tile([128, 128], f32)
                    nc.tensor.transpose(qt_ps, q_s[:, 128 * t:128 * (t + 1)], ident)
                    qt_s = qtp.tile([128, 128], f32)
                    nc.vector.tensor_copy(qt_s, qt_ps)

                    o_ps_a = ps_o.tile([128, 64], f32, tag="oa")
                    o_ps_b = ps_o.tile([128, 64], f32, tag="ob")
                    nc.tensor.matmul(o_ps_a, qt_s[0:64, :], kv_s[0:64, :],
                                     start=True, stop=True)
                    nc.tensor.matmul(o_ps_b, qt_s[64:128, :], kv_s[64:128, :],
                                     start=True, stop=True)
                    nc.vector.tensor_copy(out_s[:, 128 * t:128 * t + 64], o_ps_a)
                    nc.vector.tensor_copy(out_s[:, 128 * t + 64:128 * (t + 1)], o_ps_b)

                nc.sync.dma_start(
                    out=Od[j].rearrange("(p c) d -> p (c d)", p=128), in_=out_s
                )
    nc.compile()
    return nc


def _get_nc():
    global _NC
    if _NC is None:
        _NC = _build()
    return _NC


def kernel(Q, K, V, x):
    from concourse.bass_utils import run_bass_kernel_spmd

    Q = np.asarray(Q, dtype=np.float32)
    K = np.asarray(K, dtype=np.float32)
    V = np.asarray(V, dtype=np.float32)
    x = np.asarray(x, dtype=np.float32)

    nc = _get_nc()
    in_maps = []
    for i in range(NCORES):
        b, h0 = i // 2, (i % 2) * PAIRS
        in_maps.append({
            "q": np.ascontiguousarray(Q[b, h0:h0 + PAIRS]),
            "k": np.ascontiguousarray(K[b, h0:h0 + PAIRS]),
            "v": np.ascontiguousarray(V[b, h0:h0 + PAIRS]),
            "x": np.ascontiguousarray(x[b, 0, :, 0]),
        })

    res = run_bass_kernel_spmd(nc, in_maps, core_ids=list(range(NCORES)))

    out = np.empty((B, H, N, D), dtype=np.float32)
    for i in range(NCORES):
        b, h0 = i // 2, (i % 2) * PAIRS
        out[b, h0:h0 + PAIRS] = res.results[i]["out"]
    return out
